# revision 1
# baseline (speedup 1.0000x reference)
"""Trainium2 Bass kernel for nn_BinLoss_7103875908252.

Computes: labels = histogram-bin(target) -> combined bin id in [0, 1024);
          loss = mean_i ||features_i - centers[labels_i]||^2   (clip is a
          no-op for this data regime: sq_dist in [~500, ~900]).

Sharding: data-parallel over the batch axis across 8 NeuronCores
(4096 rows each); centers table stays in DRAM and is gathered per-row
via indirect DMA.  Each core returns a partial sum; host sums and
divides by N.

Per-core layout: row i = p*32 + r lives in partition p, slot r.
  - target  [4096, 2]  -> SBUF [128, 32, 2] (natural row-major DMA)
  - binning: count of (v > edge_j) over the 31 exact f32 edges
    (bit-exact vs jnp.searchsorted side='left' on f32)
  - label   = b0*32 + b1 -> int32 [128, 32, 1]
  - per r-slot t: gather centers rows [128, 512] via indirect DMA,
    diff = features - gathered (DVE), Square+row-accumulate (ACT)
    -> acc[:, t]
  - finish: reduce acc over free dim, 128->1 via ones matmul (PE),
    DMA the [1,1] partial sum out.
"""

import numpy as np

P = 128           # partitions
R = 32            # rows per partition per core
D = 512           # feature dim
K = 1024          # number of centers
NCORES = 8
N = 32768
SHARD = N // NCORES            # 4096
assert SHARD == P * R

# f32 bit patterns of jnp.linspace(0.0, 1.0, 31, dtype=float32)
EDGE_BITS = [
    0x00000000, 0x3d088889, 0x3d888889, 0x3dccccce, 0x3e088889, 0x3e2aaaab,
    0x3e4cccce, 0x3e6eeef0, 0x3e888889, 0x3e99999a, 0x3eaaaaab, 0x3ebbbbbc,
    0x3eccccce, 0x3edddddf, 0x3eeeeef0, 0x3f000000, 0x3f088889, 0x3f111112,
    0x3f19999a, 0x3f222223, 0x3f2aaaab, 0x3f333334, 0x3f3bbbbc, 0x3f444445,
    0x3f4cccce, 0x3f555556, 0x3f5ddddf, 0x3f666667, 0x3f6eeef0, 0x3f777778,
    0x3f800000,
]
EDGES = [float(np.uint32(b).view(np.float32)) for b in EDGE_BITS]
NE = len(EDGES)   # 31

_CACHE = {}


def build_bass():
    """Build + compile the per-core Bass/Tile kernel (SPMD, same NEFF on
    all 8 cores)."""
    from contextlib import ExitStack

    import concourse.bacc as bacc
    import concourse.tile as tile
    from concourse import bass, mybir

    f32 = mybir.dt.float32
    i32 = mybir.dt.int32

    nc = bacc.Bacc(
        "TRN2", target_bir_lowering=False, debug=False, num_devices=NCORES
    )
    feat = nc.dram_tensor("features", [SHARD, D], f32, kind="ExternalInput").ap()
    targ = nc.dram_tensor("target", [SHARD, 2], f32, kind="ExternalInput").ap()
    cent = nc.dram_tensor("centers", [K, D], f32, kind="ExternalInput").ap()
    out = nc.dram_tensor("out", [1, 1], f32, kind="ExternalOutput").ap()

    with tile.TileContext(nc) as tc, ExitStack() as ctx:
        const_p = ctx.enter_context(tc.tile_pool(name="const", bufs=1))
        work_p = ctx.enter_context(tc.tile_pool(name="work", bufs=1))
        gat_p = ctx.enter_context(tc.tile_pool(name="gat", bufs=6))
        dif_p = ctx.enter_context(tc.tile_pool(name="dif", bufs=4))
        psum_p = ctx.enter_context(tc.tile_pool(name="psum", bufs=1, space="PSUM"))

        # ---- binning prologue -------------------------------------------
        ttile = work_p.tile([P, R, 2], f32)
        nc.sync.dma_start(ttile[:], targ.rearrange("(p r) c -> p r c", p=P))

        etile = const_p.tile([P, NE], f32)
        for j, e in enumerate(EDGES):
            nc.vector.memset(etile[:, j : j + 1], e)

        # cmp[p, rc, j] = (target[p, rc] > edge[j])  as f32 0/1
        cmp = work_p.tile([P, 2 * R, NE], f32)
        tvals = ttile[:].rearrange("p r c -> p (r c)")
        nc.vector.tensor_tensor(
            out=cmp[:],
            in0=tvals.unsqueeze(2).broadcast_to([P, 2 * R, NE]),
            in1=etile[:].unsqueeze(1).broadcast_to([P, 2 * R, NE]),
            op=mybir.AluOpType.is_gt,
        )
        # bins[p, rc] = sum_j cmp  (strict count == searchsorted left)
        bins = work_p.tile([P, R, 2], f32)
        nc.vector.tensor_reduce(
            out=bins[:].rearrange("p r c -> p (r c)"),
            in_=cmp[:],
            axis=mybir.AxisListType.X,
            op=mybir.AluOpType.add,
        )
        # label = b0*32 + b1
        labf = work_p.tile([P, R, 1], f32)
        nc.vector.tensor_scalar(
            out=labf[:],
            in0=bins[:, :, 0:1],
            scalar1=float(32.0),
            scalar2=None,
            op0=mybir.AluOpType.mult,
        )
        labf2 = work_p.tile([P, R, 1], f32)
        nc.vector.tensor_tensor(
            out=labf2[:], in0=labf[:], in1=bins[:, :, 1:2], op=mybir.AluOpType.add
        )
        labi = work_p.tile([P, R, 1], i32)
        nc.vector.tensor_copy(out=labi[:], in_=labf2[:])

        # ---- features load (4 x 2MB chunks) -----------------------------
        F = work_p.tile([P, R, D], f32)
        feat_re = feat.rearrange("(p r) d -> p r d", p=P)
        CH = 8
        for c in range(R // CH):
            nc.sync.dma_start(
                F[:, c * CH : (c + 1) * CH, :], feat_re[:, c * CH : (c + 1) * CH, :]
            )

        # ---- main loop ---------------------------------------------------
        acc = work_p.tile([P, R], f32)
        for t in range(R):
            g = gat_p.tile([P, D], f32)
            nc.gpsimd.indirect_dma_start(
                out=g[:],
                out_offset=None,
                in_=cent[:, :],
                in_offset=bass.IndirectOffsetOnAxis(ap=labi[:, t, :], axis=0),
            )
            d = dif_p.tile([P, D], f32)
            nc.vector.tensor_tensor(
                out=d[:], in0=F[:, t, :], in1=g[:], op=mybir.AluOpType.subtract
            )
            nc.scalar.activation(
                out=d[:],
                in_=d[:],
                func=mybir.ActivationFunctionType.Square,
                accum_out=acc[:, t : t + 1],
            )

        # ---- final reduction --------------------------------------------
        s = work_p.tile([P, 1], f32)
        nc.vector.tensor_reduce(
            out=s[:], in_=acc[:], axis=mybir.AxisListType.X, op=mybir.AluOpType.add
        )
        ones = const_p.tile([P, 1], f32)
        nc.vector.memset(ones[:], 1.0)
        ps = psum_p.tile([1, 1], f32)
        nc.tensor.matmul(out=ps[:], lhsT=ones[:], rhs=s[:], start=True, stop=True)
        res = work_p.tile([1, 1], f32)
        nc.vector.tensor_copy(out=res[:], in_=ps[:])
        nc.sync.dma_start(out[:, :], res[:])

    nc.compile()
    return nc


def _get_nc():
    if "nc" not in _CACHE:
        _CACHE["nc"] = build_bass()
    return _CACHE["nc"]


def kernel(features, target, centers):
    from concourse.bass_utils import run_bass_kernel_spmd

    features = np.ascontiguousarray(features, dtype=np.float32)
    target = np.ascontiguousarray(target, dtype=np.float32)
    centers = np.ascontiguousarray(centers, dtype=np.float32)

    nc = _get_nc()
    in_maps = []
    for c in range(NCORES):
        sl = slice(c * SHARD, (c + 1) * SHARD)
        in_maps.append(
            {
                "features": np.ascontiguousarray(features[sl]),
                "target": np.ascontiguousarray(target[sl]),
                "centers": centers,
            }
        )
    r = run_bass_kernel_spmd(
        nc,
        in_maps,
        core_ids=list(range(NCORES)),
        trace=_CACHE.get("trace", False),
        tmpdir=_CACHE.get("tmpdir"),
    )
    _CACHE["last_results"] = r
    total = sum(float(res["out"][0, 0]) for res in r.results)
    return np.float32(total / N)
